# revision 5
# baseline (speedup 1.0000x reference)
"""Trainium2 Bass kernel for nn_NestedFeedForward (nested MoE feed-forward).

Per token, expert m in [1,4] selects active width Dm = 2048 >> (4-m):
    y[:Dm] = gelu(x[:Dm] @ w1[:, :Dm].T + b1) @ w2[:Dm].T + b2[:Dm],  y[Dm:] = 0

Strategy: sort tokens by expert on the host so per-token GEMM depth scales
with Dm, give every core an identical per-expert token count (FLOP-balanced
SPMD, one program), run fp16 tiled matmuls with fp32 PSUM accumulation and
weights fully SBUF-resident. Host gathers/transposes inputs and scatters the
(feature-major) outputs back.
"""

import math

import numpy as np

_B, _S, _D = 4, 4096, 2048
_NEXP = 4
_NCHUNK = _D // 128  # 16
_NCORES = 8
_CCH = [2, 4, 8, 16]  # k/d chunks per expert (Dm/128)
_TMAX = 512

_compiled_cache: dict = {}


def _split_tiles(p):
    """Split p columns into near-equal tiles of at most _TMAX, multiples of 4."""
    if p == 0:
        return []
    n_t = max(1, math.ceil(p / _TMAX))
    base = (p // n_t) // 4 * 4
    sizes = [base] * n_t
    rem = p - base * n_t
    i = 0
    while rem > 0:
        add = min(4, rem)
        sizes[i] += add
        rem -= add
        i = (i + 1) % n_t
    assert sum(sizes) == p and all(s <= _TMAX for s in sizes)
    return sizes


def _build(p_counts):
    """Build+compile the SPMD program for per-core per-expert counts p_counts."""
    import concourse.bacc as bacc
    import concourse.mybir as mybir
    import concourse.tile as tile

    f16 = mybir.dt.float16
    f32 = mybir.dt.float32

    # (expert m, col offset, tile width) work list; experts ascending so the
    # weight prefix an expert needs has arrived by the time its tiles run.
    tiles = []
    off = 0
    for m in range(_NEXP):
        for t in _split_tiles(p_counts[m]):
            tiles.append((m, off, t))
            off += t
    P = off

    nc = bacc.Bacc("TRN2", target_bir_lowering=False, debug=False)
    x_t = nc.dram_tensor("x_t", [_D, P], f16, kind="ExternalInput")
    w1t = nc.dram_tensor("w1t", [_D, _D], f16, kind="ExternalInput")
    w2q = nc.dram_tensor("w2q", [_D, _D], f16, kind="ExternalInput")
    b1q = nc.dram_tensor("b1q", [128, _NCHUNK], f32, kind="ExternalInput")
    b2q = nc.dram_tensor("b2q", [128, _NCHUNK], f32, kind="ExternalInput")
    y_t = nc.dram_tensor("y_t", [_D, P], f32, kind="ExternalOutput")

    gelu = mybir.ActivationFunctionType.Gelu

    with tile.TileContext(nc) as tc:
        with (
            tc.tile_pool(name="wpool", bufs=1) as wpool,
            tc.tile_pool(name="xpool", bufs=2) as xpool,
            tc.tile_pool(name="hpool", bufs=1) as hpool,
            tc.tile_pool(name="opool", bufs=4) as opool,
            tc.tile_pool(name="ps1", bufs=4, space="PSUM") as ps1pool,
            tc.tile_pool(name="ps2", bufs=4, space="PSUM") as ps2pool,
        ):
            b1_sb = wpool.tile([128, _NCHUNK], f32, name="b1sb", tag="b1")
            nc.sync.dma_start(b1_sb[:], b1q.ap())
            b2_sb = wpool.tile([128, _NCHUNK], f32, name="b2sb", tag="b2")
            nc.sync.dma_start(b2_sb[:], b2q.ap())

            # PE warmup: dummy matmuls on a zeroed tile keep the HAM activity
            # monitor busy while the first weight/x DMAs land, so real matmuls
            # start at 2.4 GHz instead of 1.2 GHz.
            warm = wpool.tile([128, 512], f16, name="warm", tag="warm")
            nc.vector.memset(warm[:], 0.0)
            for wi in range(24):
                wps = ps1pool.tile([128, 512], f32, name="wmps", tag="ps1")
                nc.tensor.matmul(
                    wps[:], warm[:, :128], warm[:], start=True, stop=True
                )

            w1_sb = [None] * _NCHUNK
            w2_sb = [None] * _NCHUNK

            def load_weights_upto(c):
                # issue DMA loads for w1/w2 chunk rows not yet resident
                for k in range(c):
                    if w1_sb[k] is None:
                        w1_sb[k] = wpool.tile(
                            [128, _D], f16, name=f"w1sb{k}", tag=f"w1_{k}"
                        )
                        nc.sync.dma_start(
                            w1_sb[k][:], w1t.ap()[k * 128 : (k + 1) * 128, :]
                        )
                for k in range(c):
                    if w2_sb[k] is None:
                        w2_sb[k] = wpool.tile(
                            [128, _D], f16, name=f"w2sb{k}", tag=f"w2_{k}"
                        )
                        nc.sync.dma_start(
                            w2_sb[k][:], w2q.ap()[k * 128 : (k + 1) * 128, :]
                        )

            for m, off, t in tiles:
                c = _CCH[m]
                load_weights_upto(c)

                xt = []
                for k in range(c):
                    xk = xpool.tile([128, _TMAX], f16, name=f"xk{k}", tag=f"x{k}")
                    # ACT HWDGE ring: keeps x loads out of the FIFO behind the
                    # bulk weight loads on the SP ring.
                    nc.scalar.dma_start(
                        xk[:, :t], x_t.ap()[k * 128 : (k + 1) * 128, off : off + t]
                    )
                    xt.append(xk)

                hs = []
                for o in range(_NCHUNK):
                    ps = ps1pool.tile([128, _TMAX], f32, name="ps1t", tag="ps1")
                    for k in range(c):
                        nc.tensor.matmul(
                            ps[:, :t],
                            w1_sb[k][:, o * 128 : (o + 1) * 128],
                            xt[k][:, :t],
                            start=(k == 0),
                            stop=(k == c - 1),
                        )
                    ho = hpool.tile([128, _TMAX], f16, name=f"ho{o}", tag=f"h{o}")
                    nc.scalar.activation(ho[:, :t], ps[:, :t], gelu, bias=b1_sb[:, o : o + 1])
                    hs.append(ho)

                for d in range(c):
                    ps2 = ps2pool.tile([128, _TMAX], f32, name="ps2t", tag="ps2")
                    for o in range(_NCHUNK):
                        nc.tensor.matmul(
                            ps2[:, :t],
                            w2_sb[d][:, o * 128 : (o + 1) * 128],
                            hs[o][:, :t],
                            start=(o == 0),
                            stop=(o == _NCHUNK - 1),
                        )
                    yo = opool.tile([128, _TMAX], f32, name="yot", tag="yo")
                    nc.vector.tensor_scalar_add(yo[:, :t], ps2[:, :t], b2_sb[:, d : d + 1])
                    # SWDGE: output stores on their own path, off both HWDGE rings
                    nc.gpsimd.dma_start(
                        y_t.ap()[d * 128 : (d + 1) * 128, off : off + t], yo[:, :t]
                    )

    nc.compile()
    return nc, P, tiles


def _get_compiled(p_counts):
    key = tuple(p_counts)
    if key not in _compiled_cache:
        _compiled_cache[key] = _build(p_counts)
    return _compiled_cache[key]


def _prep_weights(w1, b1, w2, b2):
    w1t = np.ascontiguousarray(w1.T).astype(np.float16)  # [k, o]
    # w2q row d*128+p, col oc*128+j  =  w2T[oc*128+p, d*128+j] = w2[d*128+j, oc*128+p]
    w2q = np.ascontiguousarray(
        w2.reshape(_NCHUNK, 128, _NCHUNK, 128).transpose(0, 3, 2, 1).reshape(_D, _D)
    ).astype(np.float16)
    b1q = np.ascontiguousarray(b1.reshape(_NCHUNK, 128).T).astype(np.float32)
    b2q = np.ascontiguousarray(b2.reshape(_NCHUNK, 128).T).astype(np.float32)
    return w1t, w2q, b1q, b2q


def kernel(x, w1, b1, w2, b2, token_mask):
    from concourse import bass_utils

    x = np.asarray(x, dtype=np.float32)
    w1 = np.asarray(w1, dtype=np.float32)
    b1 = np.asarray(b1, dtype=np.float32)
    w2 = np.asarray(w2, dtype=np.float32)
    b2 = np.asarray(b2, dtype=np.float32)
    tm = np.asarray(token_mask).reshape(-1)

    x_flat = x.reshape(-1, _D)
    n_tok = x_flat.shape[0]

    valid = (tm >= 1) & (tm <= _NEXP)
    expert = np.where(valid, tm - 1, -1)  # 0..3, -1 invalid

    # token index lists per expert, padded per-core-count to multiple of 4
    idx_by_exp = [np.nonzero(expert == m)[0] for m in range(_NEXP)]
    counts = [len(ix) for ix in idx_by_exp]
    p_counts = [4 * math.ceil(cnt / (4 * _NCORES)) if cnt else 0 for cnt in counts]

    nc, P, tiles_list = _get_compiled(p_counts)

    # per-core token lists (padded entries point at token 0, dropped on scatter)
    core_tok = np.zeros((_NCORES, P), dtype=np.int64)
    core_valid = np.zeros((_NCORES, P), dtype=bool)
    off = 0
    for m in range(_NEXP):
        pm = p_counts[m]
        if pm == 0:
            continue
        padded = np.zeros(pm * _NCORES, dtype=np.int64)
        padded[: counts[m]] = idx_by_exp[m]
        vmask = np.zeros(pm * _NCORES, dtype=bool)
        vmask[: counts[m]] = True
        core_tok[:, off : off + pm] = padded.reshape(_NCORES, pm)
        core_valid[:, off : off + pm] = vmask.reshape(_NCORES, pm)
        off += pm
    assert off == P

    w1t, w2q, b1q, b2q = _prep_weights(w1, b1, w2, b2)

    in_maps = []
    for j in range(_NCORES):
        x_tj = x_flat[core_tok[j]].T.astype(np.float16)  # [D, P] contiguous
        in_maps.append(
            {"x_t": x_tj, "w1t": w1t, "w2q": w2q, "b1q": b1q, "b2q": b2q}
        )

    res = bass_utils.run_bass_kernel_spmd(nc, in_maps, core_ids=list(range(_NCORES)))

    y_flat = np.zeros((n_tok, _D), dtype=np.float32)
    for j in range(_NCORES):
        yt = res.results[j]["y_t"]  # [D, P] fp32
        v = core_valid[j]
        y_flat[core_tok[j][v]] = yt[:, v].T
    return y_flat.reshape(x.shape)
